# revision 6
# baseline (speedup 1.0000x reference)
"""NT-Xent (contrastive) loss kernel for Trainium2, 8 NeuronCores — v2.

Exploits the symmetry of the similarity matrix: only ~half the 64x64 grid
of 128x128 sim tiles is computed. Tile coverage uses a cyclic scheme so one
SPMD program serves all 8 cores on per-core *rotated* inputs:

  Global tile pair {a, b} (a != b) is computed exactly once:
    - offset d = (b - a) mod 64 in [1, 31]: from row-tile a
    - d = 32: from row-tile min side (global row-tile < 32)
    - d = 0 (diagonal): from row-tile a
  Core c owns global row-tiles {c + 8*s : s in 0..7}. Its input is
  z rolled by -128*c rows, so locally every core runs the identical
  program: slot s = local row-tile 8s, sweeping local columns
  [1024*s, 1024*s + 4096) (d = 0..31, wrapped mod 8192), plus for
  s < 4 the d=32 tile at local cols [1024*s + 4096, +128).

  For each exp'd sim tile:
    - row sums   -> ACT accumulator (fused with the exp)      -> S rows
    - column sums-> elementwise accumulation into a strip A   -> S cols
      (d=0 and d=32-from-both-sides tiles skip the strip to avoid
       double counting; d=32 tiles are computed from both sides, each
       contributing its row sums only... except the single-sided tail
       tiles which contribute both.)
  Host: gathers per-core row-sum parts, strip partials (reducing the
  128 partitions in numpy), assembles S, and computes
  loss = mean(log(S - e^2) - pos).  pos comes from on-device bf16
  row-dot products (fused TT+accum).

Normalization is on-device: ssq via per-tile TT(mult)+accum, then
rn = (TEMP*ssq)^-1/2 = exp(-0.5*ln(TEMP*ssq)) on ACT, scale on DVE,
PE transposes into feature-major layout for the matmuls.
"""

import sys

import numpy as np

if "/opt/trn_rl_repo" not in sys.path:
    sys.path.insert(0, "/opt/trn_rl_repo")

TWO_N = 8192
DIM = 128
N_CORES = 8
TEMP = 0.5
NT = 64  # 128-row tiles in the global grid
SLOTS = 8  # row-tiles per core
CH = 2048  # exp chunk width (psum: 4 banks fp32)
W_MAIN = 4096  # main sweep width per slot (d = 0..31)


def _build():
    import os
    from contextlib import ExitStack

    debug_phase = os.environ.get("V2_DEBUG", "all")

    import concourse.bass as bass
    import concourse.tile as tile
    from concourse import bacc, masks, mybir

    f32 = mybir.dt.float32
    bf16 = mybir.dt.bfloat16
    AF = mybir.ActivationFunctionType
    ALU = mybir.AluOpType

    nc = bacc.Bacc("TRN2", target_bir_lowering=False, debug=False)
    z16 = nc.dram_tensor("z16", [TWO_N, DIM], bf16, kind="ExternalInput").ap()
    a_out = nc.dram_tensor("a_out", [128, TWO_N], bf16, kind="ExternalOutput").ap()
    # stats: [sparts 16 | d32r 4 | pos 8] = 28 cols
    st_out = nc.dram_tensor("stats", [128, 28], f32, kind="ExternalOutput").ap()

    with tile.TileContext(nc) as tc, ExitStack() as ctx:
        const_pool = ctx.enter_context(tc.tile_pool(name="const", bufs=1))
        ld_pool = ctx.enter_context(tc.tile_pool(name="ld", bufs=3))
        stat_pool = ctx.enter_context(tc.tile_pool(name="stat", bufs=1))
        scr_pool = ctx.enter_context(tc.tile_pool(name="scr", bufs=2))
        rows_pool = ctx.enter_context(tc.tile_pool(name="rows", bufs=1))
        psum_pool = ctx.enter_context(tc.tile_pool(name="psum", bufs=2, space="PSUM"))
        es_pool = ctx.enter_context(tc.tile_pool(name="es", bufs=2))

        identity = const_pool.tile([128, 128], bf16, tag="ident")
        masks.make_identity(nc, identity[:])

        znb = rows_pool.tile([128, TWO_N], bf16, tag="znb")
        znbT = rows_pool.tile([128, TWO_N], bf16, tag="znbT")
        astrip = rows_pool.tile([128, TWO_N], bf16, tag="astrip")

        ssq = stat_pool.tile([128, NT], f32, tag="ssq")
        lnt = stat_pool.tile([128, NT], f32, tag="lnt")
        rn = stat_pool.tile([128, NT], f32, tag="rn")
        sparts = stat_pool.tile([128, 16], f32, tag="sparts")
        d32r = stat_pool.tile([128, 4], f32, tag="d32r")
        posv = stat_pool.tile([128, 8], f32, tag="posv")

        # ---------- prep: load, ssq, rn, scale, transpose -------------
        def prep_group(g):
            """Rows [1024g, 1024(g+1)): load, per-tile ssq, (scale+transpose
            emitted separately once rn for the group exists)."""
            zt = ld_pool.tile([128, 1024], bf16, tag="ld")
            nc.sync.dma_start(
                zt[:].rearrange("p (a f) -> p a f", f=128),
                z16[g * 1024 : (g + 1) * 1024, :].rearrange("(a p) f -> p a f", p=128),
            )
            sq_scr = scr_pool.tile([128, 128], bf16, tag="sq")
            for i in range(8):
                a = 8 * g + i
                nc.vector.scalar_tensor_tensor(
                    sq_scr[:],
                    zt[:, i * 128 : (i + 1) * 128],
                    1.0,
                    zt[:, i * 128 : (i + 1) * 128],
                    ALU.mult,
                    ALU.mult,
                    accum_out=ssq[:, a : a + 1],
                )
            return zt

        def rn_batch(g0, g1):
            """rn for groups [g0, g1): rn = exp(-0.5 * ln(TEMP * ssq))."""
            sl = slice(8 * g0, 8 * g1)
            nc.scalar.activation(lnt[:, sl], ssq[:, sl], AF.Ln, scale=float(TEMP))
            nc.scalar.activation(rn[:, sl], lnt[:, sl], AF.Exp, scale=-0.5)

        def scale_transpose_group(g, zt):
            """znb tiles for group g, then PE-transpose them into znbT."""
            for i in range(8):
                a = 8 * g + i
                nc.vector.tensor_scalar_mul(
                    znb[:, a * 128 : (a + 1) * 128],
                    zt[:, i * 128 : (i + 1) * 128],
                    rn[:, a : a + 1],
                )
            pt = psum_pool.tile([128, 1024], bf16, tag="mm")
            for i in range(8):
                a = 8 * g + i
                nc.tensor.transpose(
                    pt[:, i * 128 : (i + 1) * 128],
                    znb[:, a * 128 : (a + 1) * 128],
                    identity[:],
                )
            nc.vector.tensor_copy(znbT[:, g * 1024 : (g + 1) * 1024], pt[:])

        # Pipeline: batches of 2 groups (ld_pool bufs=3 -> max 2 live + 1
        # inflight).
        for g0 in range(0, 8, 2):
            za = prep_group(g0)
            zb = prep_group(g0 + 1)
            rn_batch(g0, g0 + 2)
            scale_transpose_group(g0, za)
            scale_transpose_group(g0 + 1, zb)

        # ---------- strip bookkeeping (compile-time) ------------------
        covered = [False] * NT  # per 128-col tile of astrip
        last_strip = {}  # tile -> emitted after which (s, k)

        def strip_ranges(c0, c1):
            """Normalize local [c0, c1) mod TWO_N, split at the wrap seam."""
            n = c1 - c0
            c0 = c0 % TWO_N
            if c0 + n <= TWO_N:
                return [(c0, c0 + n)]
            return [(c0, TWO_N), (0, c0 + n - TWO_N)]

        def emit_strip(es_t, es_off, c0, c1):
            """astrip[:, c0:c1) (+)= es_t[:, es_off:...]. Splits into
            copy (first touch) / add runs at tile granularity."""
            t0, t1 = c0 // 128, c1 // 128
            i = t0
            while i < t1:
                j = i
                state = covered[i]
                while j < t1 and covered[j] == state:
                    j += 1
                lo, hi = i * 128, j * 128
                eo = es_off + (lo - c0)
                if state:
                    nc.vector.tensor_tensor(
                        astrip[:, lo:hi],
                        astrip[:, lo:hi],
                        es_t[:, eo : eo + hi - lo],
                        ALU.add,
                    )
                else:
                    nc.vector.tensor_copy(
                        astrip[:, lo:hi], es_t[:, eo : eo + hi - lo]
                    )
                    for t in range(i, j):
                        covered[t] = True
                i = j

        # ---------- main loop -----------------------------------------
        do_main = debug_phase != "prep"
        do_strips = debug_phase not in ("main", "prep")
        do_tail = debug_phase == "all"
        if debug_phase == "prep":
            nc.vector.memset(astrip[:], 0.0)
        # slot s: weights = znbT tile 8s; cols [1024s + 2048k, +2048).
        for s in range(SLOTS if do_main else 0):
            wsl = znbT[:, 1024 * s : 1024 * s + 128]
            for k in range(2):
                c0 = 1024 * s + CH * k
                pt = psum_pool.tile([128, CH], f32, tag="mm")
                off = 0
                for r0, r1 in strip_ranges(c0, c0 + CH):
                    n = r1 - r0
                    for q0 in range(0, n, 512):
                        qn = min(512, n - q0)
                        nc.tensor.matmul(
                            pt[:, off + q0 : off + q0 + qn],
                            lhsT=wsl,
                            rhs=znbT[:, r0 + q0 : r0 + q0 + qn],
                            start=True,
                            stop=True,
                        )
                    off += n
                es_t = es_pool.tile([128, CH], bf16, tag="es")
                nc.scalar.activation(
                    es_t[:],
                    pt[:],
                    AF.Exp,
                    accum_out=sparts[:, 2 * s + k : 2 * s + k + 1],
                )
                # strips: skip the d=0 (diag) tile = first 128 cols of k=0
                if do_strips:
                    sc0 = c0 + (128 if k == 0 else 0)
                    es_off = 128 if k == 0 else 0
                    for r0, r1 in strip_ranges(sc0, c0 + CH):
                        emit_strip(es_t, es_off, r0, r1)
                        es_off += r1 - r0

        # ---------- tail: pos, d32, outputs ---------------------------
        if not do_tail:
            if do_main and not do_strips:
                nc.vector.memset(astrip[:], 0.0)
            nc.vector.memset(sparts[:] if not do_main else d32r[:], 0.0)
            nc.vector.memset(d32r[:], 0.0)
            nc.vector.memset(posv[:], 0.0)
        pos_scr = scr_pool.tile([128, 128], bf16, tag="sq")
        for s in range(8 if do_tail else 0):
            a, b = 8 * s, (8 * s + 32) % NT
            nc.vector.scalar_tensor_tensor(
                pos_scr[:],
                znb[:, a * 128 : (a + 1) * 128],
                1.0,
                znb[:, b * 128 : (b + 1) * 128],
                ALU.mult,
                ALU.mult,
                accum_out=posv[:, s : s + 1],
            )

        for s in range(4 if do_tail else 0):
            pd = psum_pool.tile([128, 128], f32, tag="mm")
            cc = 1024 * s + 4096
            nc.tensor.matmul(
                pd[:],
                lhsT=znbT[:, 1024 * s : 1024 * s + 128],
                rhs=znbT[:, cc : cc + 128],
                start=True,
                stop=True,
            )
            ed = es_pool.tile([128, 128], bf16, tag="es")
            nc.scalar.activation(ed[:], pd[:], AF.Exp, accum_out=d32r[:, s : s + 1])
            emit_strip(ed, 0, cc, cc + 128)

        nc.sync.dma_start(a_out, astrip[:])
        nc.sync.dma_start(st_out[:, 0:16], sparts[:])
        nc.sync.dma_start(st_out[:, 16:20], d32r[:])
        nc.sync.dma_start(st_out[:, 20:28], posv[:])

    # Force Ln and Exp onto the single shared ACT table set.
    import concourse.bacc as bacc_mod
    from concourse.hw_specs import get_activation_tables as _real_gat

    def _gat_ln_exp_shared(arch):
        tabs = _real_gat(arch)
        out = {}
        for name, fns in tabs.items():
            if name != "natural_log_exp_and_others":
                fns = fns - {AF.Ln, AF.Exp}
            out[name] = fns
        return out

    bacc_mod.get_activation_tables = _gat_ln_exp_shared
    try:
        nc.compile()
    finally:
        bacc_mod.get_activation_tables = _real_gat
    return nc


_NC_CACHE = None


def _get_nc():
    global _NC_CACHE
    if _NC_CACHE is None:
        _NC_CACHE = _build()
    return _NC_CACHE


def make_in_maps(z_i: np.ndarray, z_j: np.ndarray):
    import ml_dtypes

    z = np.concatenate([z_i, z_j], axis=0).astype(np.float32)
    z16 = z.astype(ml_dtypes.bfloat16)
    in_maps = []
    for c in range(N_CORES):
        in_maps.append({"z16": np.roll(z16, -128 * c, axis=0)})
    return in_maps


def assemble(results):
    """Host-side reduction: build S and pos, return the scalar loss."""
    S = np.zeros(TWO_N, dtype=np.float64)
    pos = np.zeros(TWO_N, dtype=np.float64)
    for c, r in enumerate(results):
        colsum = r["a_out"].astype(np.float64).sum(axis=0)  # [8192] local cols
        stats = r["stats"].astype(np.float64)
        sparts, d32r, posc = stats[:, 0:16], stats[:, 16:20], stats[:, 20:28]
        # local col l -> global (l + 128c) mod 8192
        S += np.roll(colsum, 128 * c)
        # local row 1024s + p -> global (1024s + p + 128c) mod 8192
        rows = np.zeros(TWO_N, dtype=np.float64)
        prow = np.zeros(TWO_N, dtype=np.float64)
        for s in range(8):
            rows[1024 * s : 1024 * s + 128] = sparts[:, 2 * s] + sparts[:, 2 * s + 1]
            if s < 4:
                rows[1024 * s : 1024 * s + 128] += d32r[:, s]
            prow[1024 * s : 1024 * s + 128] = posc[:, s]
        S += np.roll(rows, 128 * c)
        pos += np.roll(prow, 128 * c)
    lse = np.log(S - np.exp(2.0))
    return np.float32((lse - pos).mean()), S, pos


def kernel(z_i: np.ndarray, z_j: np.ndarray) -> np.ndarray:
    from concourse.bass_utils import run_bass_kernel_spmd

    nc = _get_nc()
    in_maps = make_in_maps(np.asarray(z_i), np.asarray(z_j))
    res = run_bass_kernel_spmd(nc, in_maps, core_ids=list(range(N_CORES)))
    loss, _, _ = assemble(res.results)
    return loss
